# revision 5
# baseline (speedup 1.0000x reference)
"""CQAttention Trainium2 kernel, v2: fp8e4 DoubleRow matmuls throughout.

Problem per core (one batch element): Lc=2048, Lq=512, d=512.
  S[b,i,j] = C_i.wc + Q_j.wq + sum_k wm_k C_ik Q_jk + b
  Sq = softmax_j(S); Sc = softmax_i(S)
  A  = Sq @ Q;  Bm = Sq @ (Sc^T @ C)
  out = [C | A | C*A | C*Bm]   -> [B, Lc, 4d]

Math restructuring (no bias-augmentation matmuls needed):
  All softmax normalizations are scale-invariant per their reduction axis, so
  each exp only needs the bias term that is per-PARTITION in its layout:
    En[i,j] = exp(base[i,j] + c_i - sh1)    (natural; c_i per-partition)
    Et[j,i] = exp(base[j,i]^T + qb_j - sh2) (transposed; qb_j per-partition)
  A  = (Et^T-contract @ Q) / (Et^T-contract @ 1)   (c_i, qb_j factors cancel)
  Sc-avg = (En^T @ C) / (En^T @ 1)                 (qb_j factor cancels)
  Bm = (Et^T-contract @ Sc-avg) / (same row sums)
  sh1/sh2 are per-batch host-computed shifts keeping exp <= ~100 (fp8e4 max
  normal is 240); they cancel in the normalizations.

All five matmul phases run as fp8e4 DoubleRow (0.5 cyc/row, 2 K-tiles per
instruction; operands laid out as [128, 2, M] pairs).  Score operands are
variance-balanced: X = C*sqrt(|wm|), Y = Q*sqrt(|wm|)*sign(wm) so both sides
quantize at ~0.21 sigma.  exp outputs fp8 directly from the scalar engine.
Outputs are written bf16 as two fused [128, 1024] tiles per row-tile
((C|A) and (C*A|C*Bm)) for 2 KiB DMA rows; host converts back to f32.
"""

import numpy as np

_B, _LC, _LQ, _D = 8, 2048, 512, 512
_P = 128


def _ensure_import():
    try:
        import concourse.bass  # noqa: F401
    except ImportError:
        import sys

        for p in ("/opt/trn_rl_repo", "/root/.axon_site/_ro/trn_rl_repo"):
            if p not in sys.path:
                sys.path.insert(0, p)
        import concourse.bass  # noqa: F401


def build_program(Lc=_LC, Lq=_LQ, D=_D):
    _ensure_import()
    from contextlib import ExitStack

    import concourse.mybir as mybir
    from concourse import bacc
    from concourse.tile import TileContext

    f32 = mybir.dt.float32
    f8 = mybir.dt.float8e4
    bf16 = mybir.dt.bfloat16
    EXP = mybir.ActivationFunctionType.Exp
    DR = mybir.MatmulPerfMode.DoubleRow
    MUL = mybir.AluOpType.mult
    P = _P
    NLc, NLq, ND = Lc // P, Lq // P, D // P  # 16, 4, 4
    TI, TJ, TD = NLc // 2, NLq // 2, ND // 2  # 8, 2, 2
    CH = 512  # Lc chunk for the transposed score
    NCH = Lc // CH  # 4

    nc = bacc.Bacc()
    dX = nc.declare_dram_parameter("Xdr", [P, 2 * TD, Lc], f8, isOutput=False)
    dY = nc.declare_dram_parameter("Ydr", [P, 2 * TD, Lq], f8, isOutput=False)
    dQ = nc.declare_dram_parameter("Qdr", [P, 2 * TJ, D], f8, isOutput=False)
    dC8 = nc.declare_dram_parameter("Cdr", [P, 2 * TI, D], f8, isOutput=False)
    dCbf = nc.declare_dram_parameter("Cbf", [P, NLc, D], bf16, isOutput=False)
    dcb = nc.declare_dram_parameter("c_cols", [P, NLc], f32, isOutput=False)
    dqb = nc.declare_dram_parameter("qb_cols", [P, NLq], f32, isOutput=False)
    dout = nc.declare_dram_parameter("out", [Lc, 4 * D], bf16, isOutput=True)

    with ExitStack() as ctx:
        tc = ctx.enter_context(TileContext(nc))
        sb = ctx.enter_context(tc.tile_pool(name="persist", bufs=1))
        pbig = ctx.enter_context(tc.tile_pool(name="pbig", bufs=6, space="PSUM"))
        psm = ctx.enter_context(tc.tile_pool(name="psm", bufs=2, space="PSUM"))
        stage = ctx.enter_context(tc.tile_pool(name="stage", bufs=4))

        # ---- persistent SBUF tiles ----
        tX = [sb.tile([P, 2, Lc], f8, tag=f"X{t}", name=f"X{t}") for t in range(TD)]
        tY = [sb.tile([P, 2, Lq], f8, tag=f"Y{t}", name=f"Y{t}") for t in range(TD)]
        tQ = [sb.tile([P, 2, D], f8, tag=f"Q{t}", name=f"Q{t}") for t in range(TJ)]
        tC8 = [sb.tile([P, 2, D], f8, tag=f"C8{t}", name=f"C8{t}") for t in range(TI)]
        tEn = [sb.tile([P, 2, Lq], f8, tag=f"En{t}", name=f"En{t}") for t in range(TI)]
        tEt = [sb.tile([P, 2, Lc], f8, tag=f"Et{t}", name=f"Et{t}") for t in range(TJ)]
        tSc = [sb.tile([P, 2, D], f8, tag=f"Sc{t}", name=f"Sc{t}") for t in range(TJ)]
        tOA = [
            sb.tile([P, 2 * D], bf16, tag=f"OA{i}", name=f"OA{i}") for i in range(NLc)
        ]
        tcb = sb.tile([P, NLc], f32, name="cbias")
        tqb = sb.tile([P, NLq], f32, name="qbias")
        tones = sb.tile([P, 2, 1], f8, name="ones8")
        twj = sb.tile([P, 2, 512], f8, name="warmjunk")
        trr = [sb.tile([P, 1], f32, tag=f"rr{i}", name=f"rr{i}") for i in range(NLc)]
        tcsr = [sb.tile([P, 1], f32, tag=f"cs{j}", name=f"cs{j}") for j in range(NLq)]

        # ---- input DMA (ordered so early phases' operands land first) ----
        nc.vector.memset(tones[:], 1.0)
        nc.vector.memset(twj[:], 0.25)
        nc.sync.dma_start(out=tcb[:], in_=dcb[:, :])
        nc.sync.dma_start(out=tqb[:], in_=dqb[:, :])
        for t in range(TD):
            nc.sync.dma_start(out=tY[t][:], in_=dY[:, 2 * t : 2 * t + 2, :])
        for t in range(TD):
            nc.sync.dma_start(out=tX[t][:], in_=dX[:, 2 * t : 2 * t + 2, :])
        for t in range(TI):
            nc.sync.dma_start(out=tC8[t][:], in_=dC8[:, 2 * t : 2 * t + 2, :])
        for t in range(TJ):
            nc.sync.dma_start(out=tQ[t][:], in_=dQ[:, 2 * t : 2 * t + 2, :])
        for i in range(NLc):
            nc.sync.dma_start(out=tOA[i][:, 0:D], in_=dCbf[:, i, :])

        # ---- PE warmup on memset tiles (no DMA dependency): lift the HAM
        # clock-gate while the score operands stream in.
        warm_ps = pbig.tile([P, 512], f32, tag="ps", name="warm_ps")
        for _w in range(10):
            nc.tensor.matmul(
                warm_ps[:], twj[:, :, 0:P], twj[:], start=True, stop=True, perf_mode=DR
            )

        # ---- natural score + exp -> En (fp8) ----
        for i in range(NLc):
            ps = pbig.tile([P, Lq], f32, tag="ps", name=f"psn{i}")
            for t in range(TD):
                nc.tensor.matmul(
                    ps[:],
                    tX[t][:, :, i * P : (i + 1) * P],
                    tY[t][:],
                    start=(t == 0),
                    stop=(t == TD - 1),
                    perf_mode=DR,
                )
            nc.scalar.activation(
                tEn[i // 2][:, i % 2, :], ps[:], EXP, bias=tcb[:, i : i + 1]
            )

        # ---- transposed score + exp -> Et (fp8), chunk-outer ----
        for n in range(NCH):
            for j in range(NLq):
                ps = pbig.tile([P, CH], f32, tag="ps", name=f"pst{n}_{j}")
                for t in range(TD):
                    nc.tensor.matmul(
                        ps[:],
                        tY[t][:, :, j * P : (j + 1) * P],
                        tX[t][:, :, n * CH : (n + 1) * CH],
                        start=(t == 0),
                        stop=(t == TD - 1),
                        perf_mode=DR,
                    )
                nc.scalar.activation(
                    tEt[j // 2][:, j % 2, n * CH : (n + 1) * CH],
                    ps[:],
                    EXP,
                    bias=tqb[:, j : j + 1],
                )

        # ---- gap filler: keep the PE stream alive while the scalar engine
        # finishes the natural exps P5 depends on (HAM drops to half clock
        # after ~a few us of PE idle).
        for _w in range(14):
            nc.tensor.matmul(
                warm_ps[:], twj[:, :, 0:P], twj[:], start=True, stop=True, perf_mode=DR
            )

        # ---- P5: Sc-weighted context average -> tSc (fp8) ----
        for j in range(NLq):
            psF = pbig.tile([P, D], f32, tag="ps", name=f"psf{j}")
            psC = psm.tile([P, 1], f32, tag="psc", name=f"psc{j}")
            for t in range(TI):
                sl = tEn[t][:, :, j * P : (j + 1) * P]
                nc.tensor.matmul(
                    psF[:], sl, tC8[t][:], start=(t == 0), stop=(t == TI - 1),
                    perf_mode=DR,
                )
                nc.tensor.matmul(
                    psC[:], sl, tones[:], start=(t == 0), stop=(t == TI - 1),
                    perf_mode=DR,
                )
            nc.vector.reciprocal(tcsr[j][:], psC[:])
            nc.vector.tensor_scalar_mul(tSc[j // 2][:, j % 2, :], psF[:], tcsr[j][:])

        # ---- P6: A = row-normalized E @ Q; emit (C|A) output tiles ----
        for i in range(NLc):
            psA = pbig.tile([P, D], f32, tag="ps", name=f"psa{i}")
            psR = psm.tile([P, 1], f32, tag="psc", name=f"psr{i}")
            for t in range(TJ):
                sl = tEt[t][:, :, i * P : (i + 1) * P]
                nc.tensor.matmul(
                    psA[:], sl, tQ[t][:], start=(t == 0), stop=(t == TJ - 1),
                    perf_mode=DR,
                )
                nc.tensor.matmul(
                    psR[:], sl, tones[:], start=(t == 0), stop=(t == TJ - 1),
                    perf_mode=DR,
                )
            nc.vector.reciprocal(trr[i][:], psR[:])
            # A-scales all on DVE: the scalar engine goes straight from its
            # exps to the P7 Bm-scales, so psB frees are never queued behind
            # A-scale backlog.
            nc.vector.tensor_scalar_mul(tOA[i][:, D : 2 * D], psA[:], trr[i][:])
            nc.sync.dma_start(out=dout[i * P : (i + 1) * P, 0 : 2 * D], in_=tOA[i][:])

        # ---- P7: Bm, then (C*A | C*Bm) output tiles ----
        for i in range(NLc):
            psB = pbig.tile([P, D], f32, tag="ps", name=f"psb{i}")
            for t in range(TJ):
                nc.tensor.matmul(
                    psB[:],
                    tEt[t][:, :, i * P : (i + 1) * P],
                    tSc[t][:],
                    start=(t == 0),
                    stop=(t == TJ - 1),
                    perf_mode=DR,
                )
            tOB = stage.tile([P, 2 * D], bf16, tag="OB", name=f"OB{i}")
            # Bm-scale on the scalar engine (idle after the exps); the two
            # bf16 multiplies then run in DVE 2x mode (414ns vs a 745ns
            # psum-sourced scalar_tensor_tensor), so DVE stops pacing the
            # output tail.
            tBm = stage.tile([P, D], bf16, tag="BM", name=f"Bm{i}")
            nc.scalar.activation(
                tBm[:], psB[:], mybir.ActivationFunctionType.Copy, scale=trr[i][:]
            )
            nc.vector.tensor_mul(tOB[:, D : 2 * D], tOA[i][:, 0:D], tBm[:])
            nc.vector.tensor_mul(tOB[:, 0:D], tOA[i][:, 0:D], tOA[i][:, D : 2 * D])
            nc.sync.dma_start(
                out=dout[i * P : (i + 1) * P, 2 * D : 4 * D], in_=tOB[:]
            )

    nc.finalize()
    return nc


def prepare_in_maps(C, Q, Wo_w, Wo_b):
    """Shard over batch; per batch build fp8 DoubleRow-paired layouts."""
    import ml_dtypes

    F8 = ml_dtypes.float8_e4m3
    BF = ml_dtypes.bfloat16
    D = C.shape[-1]
    P = _P
    Lc, Lq = C.shape[1], Q.shape[1]
    NLc, NLq = Lc // P, Q.shape[1] // P
    w = np.asarray(Wo_w, np.float32)[0]
    wc, wq, wm = w[:D], w[D : 2 * D], w[2 * D :]
    b0 = np.float32(np.asarray(Wo_b, np.float32)[0])
    s = np.sqrt(np.abs(wm)).astype(np.float32)
    sy = (s * np.sign(wm)).astype(np.float32)
    LOG100 = np.float32(np.log(100.0))

    def pair_kt(a):
        # [K, M] -> [P, K//P, M] stacking K-tiles along a middle axis
        K, M = a.shape
        return np.ascontiguousarray(a.reshape(K // P, P, M).transpose(1, 0, 2))

    in_maps = []
    for b in range(C.shape[0]):
        Cb = np.ascontiguousarray(C[b], np.float32)
        Qb = np.ascontiguousarray(Q[b], np.float32)
        X8 = (Cb * s).astype(F8)  # [Lc, D]
        Y8 = (Qb * sy).astype(F8)  # [Lq, D]
        base = X8.astype(np.float32) @ Y8.astype(np.float32).T  # [Lc, Lq]
        c = (Cb @ wc).astype(np.float32)
        qbv = (Qb @ wq + b0).astype(np.float32)
        sh1 = np.float32((base + c[:, None]).max())
        sh2 = np.float32((base + qbv[None, :]).max())
        # lhsT layouts: X^T [D, Lc], Y^T [D, Lq] paired over D
        in_maps.append(
            {
                "Xdr": pair_kt(np.ascontiguousarray(X8.T)),
                "Ydr": pair_kt(np.ascontiguousarray(Y8.T)),
                "Qdr": pair_kt(Qb.astype(F8)),  # [Lq, D] paired over Lq
                "Cdr": pair_kt(Cb.astype(F8)),  # [Lc, D] paired over Lc
                "Cbf": np.ascontiguousarray(
                    Cb.astype(BF).reshape(NLc, P, D).transpose(1, 0, 2)
                ),
                "c_cols": np.ascontiguousarray(
                    (c - sh1 + LOG100).reshape(NLc, P).T
                ),
                "qb_cols": np.ascontiguousarray(
                    (qbv - sh2 + LOG100).reshape(NLq, P).T
                ),
            }
        )
    return in_maps


_prog_cache = {}


def _get_program():
    if "nc" not in _prog_cache:
        _prog_cache["nc"] = build_program()
    return _prog_cache["nc"]


def run(C, Q, Wo_w, Wo_b, **spmd_kwargs):
    """Run on hardware; returns (out [B,Lc,4d] float32, BassKernelResults)."""
    _ensure_import()
    from concourse.bass_utils import run_bass_kernel_spmd

    nc = _get_program()
    in_maps = prepare_in_maps(C, Q, Wo_w, Wo_b)
    res = run_bass_kernel_spmd(nc, in_maps, list(range(len(in_maps))), **spmd_kwargs)
    out = np.stack(
        [np.asarray(res.results[i]["out"], np.float32) for i in range(len(in_maps))],
        axis=0,
    )
    return out, res


def kernel(C, Q, Wo_w, Wo_b):
    out, _ = run(C, Q, Wo_w, Wo_b)
    return out
